# revision 44
# baseline (speedup 1.0000x reference)
"""BERT self-attention (B=4, S=1024, D=1024, H=16) on 8 TRN2 NeuronCores.

Sharding: tensor-parallel over heads. Core c owns output dims
[c*128, (c+1)*128) of Wq/Wk/Wv (= heads 2c and 2c+1) and computes those
heads' attention for all 4 batches. seq is replicated; the host pre-tiles
seqT -> [128, B, KT, S] fp16 (a k-pair part of one batch is 4KB
contiguous per partition - the widest DMA lines this layout allows) and
weight shards -> [128, KT, 128].

Per-core pipeline (per batch):
  qT/kT/vT [128, S] = W_shard @ seqT_b        (K=1024, N=512 chunks)
  v token-major via ONE 128-wide PE transpose per key block (both heads
      at once) into [v_h0 | v_h1] tiles - exactly the transpose output
      layout, so the drain is a single DVE copy
  scores: per t8 (128-key block) a QUAD of matmuls alternating head row
      groups (h0 rows 0:64 / h1 rows 64:128) back-to-back, so the PE
      runs both heads' K=64 matmuls concurrently in disjoint row tiles
      (2x throughput vs sequential).
      scoresT[j,i] = k_j . q_i  ->  expT = exp(0.125*scoresT)  (ACT)
      written into the per-batch ex tile [128, KT, HPC, 1024] fp16.
  p@v: per chunk BOTH heads run concurrently as M=64 matmuls in
      disjoint PE column groups (cols 0:64 / 64:128) accumulating into
      one [128, 512] PSUM tile (col-tiling pairs small-M matmuls the
      same way row-tiling pairs the score quads).
  softmax denominators: a 4x column-tiled quad of M=1 ones-matmuls
      (one per (head, chunk), output partitions 0/32/64/96 of one PSUM
      bank) accumulated over key blocks; drained by per-(head,chunk)
      DVE copy + reciprocal and a GpSimd partition-broadcast, all
      emitted at PERIOD START so the broadcasts are long done when the
      p@v multiplies need them. The final multiply reads the p@v tile
      straight from PSUM, deferred one chain so the broadcast wait
      never blocks the next chain's PSUM release; fp16 DMA out.

Schedule (the ACT engine carries ~68us of exp; the PE ~107us of
matmul+ldweights, so the PE is the pacer and must never stall):
  - per-BATCH seq DMA, two batches prefetched ahead (batch 0 split
    across the sync and scalar DMA rings), so projection chains never
    wait on DMA mid-period;
  - 18 throwaway matmuls at t=0 trip the HAM activity monitor toward
    the 2.4 GHz clock while batch 0's seq is still in flight;
  - batch-0 q + k(first chunk) accumulate part-by-part as the DMAs
    land; k's second chunk (not needed until score quad t8=4) is
    deferred into the first period's filler, shortening the path to
    the first exp;
  - all non-score PE work is emitted as consecutive accumulation
    chains spread between score quads by cycle weight; periods carry
    {next batch's QKV, this batch's v-transposes, previous batch's
    p@v} so each stays just above the ACT pace.

HW-validated pitfalls baked in: DVE reciprocal must read SBUF at
partition 0 (PSUM or offset-partition inputs give garbage); GpSimd
partition_broadcast only handles base-partition-0 source AND
destination; DVE access patterns cannot stride the partition dim; each
partition-disjoint accumulation group needs its own start=True (the
has_written clear is per-partition).

The host transposes the gathered [head, d, token] result back to
[token, d] (layout only - all FLOPs happen on-device).

The softmax skips the max-subtraction: scores ~ N(0,1) so exp() is
comfortably in fp32 range, and exp(x)/sum(exp(x)) is algebraically
identical to the max-shifted form.
"""

import numpy as np
from contextlib import ExitStack

import concourse.bass as bass
import concourse.tile as tile
from concourse import bacc, mybir
from concourse.bass_utils import run_bass_kernel_spmd

N_CORES = 8
B, S, D = 4, 1024, 1024
DPC = 128  # output dims per core (2 heads x 64)
HPC = 2  # heads per core
DV = 64  # head dim
KT = D // 128  # contraction tiles
NCH = S // 512  # 512-wide free-dim chunks per batch
F32 = mybir.dt.float32
F16 = mybir.dt.float16
EXP = mybir.ActivationFunctionType.Exp

# test.py may flip these to profile; the grading path leaves them alone.
TRACE = False
TRACE_KWARGS = {}
LAST_RESULTS = None

_CACHE = {}


def _emit(ctx, tc, seqT, wT, bias, ident, outcT):
    nc = tc.nc

    singles = ctx.enter_context(tc.tile_pool(name="singles", bufs=1))
    seq_pool = ctx.enter_context(tc.tile_pool(name="seq", bufs=3))
    qkv_pool = ctx.enter_context(tc.tile_pool(name="qkv", bufs=2))
    ex_pool = ctx.enter_context(tc.tile_pool(name="expT", bufs=2))
    small_pool = ctx.enter_context(tc.tile_pool(name="small", bufs=2))
    bc_pool = ctx.enter_context(tc.tile_pool(name="bc", bufs=2))
    out_pool = ctx.enter_context(tc.tile_pool(name="out", bufs=2))
    psum_ch = ctx.enter_context(tc.tile_pool(name="psum_ch", bufs=3, space="PSUM"))
    psum_sc = ctx.enter_context(tc.tile_pool(name="psum_sc", bufs=2, space="PSUM"))
    psum_den = ctx.enter_context(tc.tile_pool(name="psum_den", bufs=1, space="PSUM"))

    w_sb = {}
    b_sb = {}

    def load_w(name, eng=None):
        # one DMA per weight: DRAM [128, KT, 128] -> SBUF [128, KT, 128]
        eng = eng if eng is not None else nc.gpsimd
        wt = singles.tile([128, KT, 128], F16, tag=f"w{name}", name=f"w{name}_sb")
        eng.dma_start(wt[:], wT[name][:])
        w_sb[name] = wt
        bt = singles.tile([128, 1], F32, tag=f"b{name}", name=f"b{name}_sb")
        nc.gpsimd.dma_start(bt[:], bias[name][:])
        b_sb[name] = bt

    all_ex = {}
    qkvT_by_b = {}

    def alloc_seq(b):
        # 4 sub-tiles of 2 k-tiles each so the first QKV matmuls only wait
        # on a quarter of the batch's k-planes
        return [
            seq_pool.tile([128, 2, S], F16, tag=f"seqT{j}", name=f"seqT_b{b}p{j}")
            for j in range(4)
        ]

    def emit_dma(b, split=False):
        sq = alloc_seq(b)
        for j in range(4):
            # scalar's DMA ring is idle in the prologue; splitting batches
            # 0/1 across two rings halves their latency. Later batches stay
            # off scalar so DMA issue never delays an ACT.
            eng = nc.scalar if split and j >= 2 else nc.sync
            eng.dma_start(sq[j][:], seqT[:, b, 2 * j : 2 * j + 2, :])
        return sq

    def qkv_chains(b, sq, names=("q", "k", "v")):
        """One chain per (name, chunk): 8 consecutive matmuls accumulating
        K=1024 into one PSUM tile, then a DVE bias-add drain."""
        chains = []
        dsts = qkvT_by_b.setdefault(b, {})
        for name in names:
            dst = qkv_pool.tile([128, S], F16, tag=f"{name}T", name=f"{name}T_b{b}")
            dsts[name] = dst
            for ic in range(NCH):

                def chain(name=name, ic=ic, dst=dst):
                    ps = psum_ch.tile([128, 512], F32, tag="ch", name=f"mm_{name}{b}{ic}")
                    for kk in range(KT):
                        nc.tensor.matmul(
                            ps[:],
                            w_sb[name][:, kk, :],
                            sq[kk // 2][:, kk % 2, ic * 512 : (ic + 1) * 512],
                            start=(kk == 0),
                            stop=(kk == KT - 1),
                        )
                    nc.vector.tensor_scalar_add(
                        dst[:, ic * 512 : (ic + 1) * 512], ps[:], b_sb[name][:]
                    )

                chains.append((chain, KT * 512))
        return chains

    def vtr_chains(b):
        """v (token-major) via ONE 128-wide PE transpose per key block
        (both heads at once); DVE drains into the ones-augmented tiles."""
        chains = []
        vau = va_sets[b % 3]
        for t8 in range(KT):

            def tr(t8=t8, vau=vau):
                vT = qkvT_by_b[b]["v"]
                va = vau[t8]
                pt = psum_ch.tile([128, 128], F16, tag="ch", name=f"vtr_{b}{t8}")
                nc.tensor.transpose(
                    pt[:], vT[:, t8 * 128 : (t8 + 1) * 128], id_sb[:]
                )
                nc.vector.tensor_copy(va[:], pt[:])

            chains.append((tr, 192))
        return chains

    def pv_chains(b, final=False, tail=False):
        """Per chunk, BOTH heads' p@v run concurrently as M=64 matmuls in
        disjoint PE column groups (cols 0:64 / 64:128) accumulating into one
        [128, 512] PSUM tile. The softmax denominators come from a separate
        4x column-tiled quad of M=1 ones-matmuls (one per (head, chunk), at
        output partitions 0/32/64/96 of one PSUM bank), drained by 4 ACT
        copies (ScalarE has slack), one DVE reciprocal over [4, 512], and a
        GpSimd partition-broadcast per (head, chunk). The final multiply is
        one [128, 512] DVE op reading the PV tile straight from PSUM,
        deferred one chain so the broadcast wait never blocks the next
        chain's PSUM release; fp16 DMA out."""
        chains = []
        vau = va_sets[b % 3]
        deferred = []
        out_eng = nc.sync if b == B - 1 else nc.gpsimd
        HIC = [(0, 0), (1, 0), (0, 1), (1, 1)]
        bct_by_hic = {}

        state = {}

        def den_chain(t8s=range(KT), last=True):
            ex = all_ex[b]
            if "dps" not in state:
                state["dps"] = psum_den.tile(
                    [128, 512], F32, tag="den", name=f"denps_{b}")
            dps = state["dps"]
            for t8 in t8s:
                for j, (h, ic) in enumerate(HIC):
                    nc.tensor.matmul(
                        dps[32 * j : 32 * j + 1, :],
                        ones_sb[:, 0:1],
                        ex[:, t8, h, ic * 512 : (ic + 1) * 512],
                        start=(t8 == 0),
                        stop=(t8 == KT - 1),
                        skip_group_check=True,
                        tile_position=(0, 32 * j),
                    )
            if not last:
                return
            for j, (h, ic) in enumerate(HIC):
                den_sb = small_pool.tile([1, 512], F32, tag=f"den{j}",
                                         name=f"den_{b}{j}")
                nc.vector.tensor_copy(den_sb[:], dps[32 * j : 32 * j + 1, :])
                rc = small_pool.tile([1, 512], F32, tag=f"recip{j}",
                                     name=f"rc_{b}{j}")
                nc.vector.reciprocal_approx_fast(rc[:], den_sb[:])
                bct = bc_pool.tile([DV, 512], F32, tag=f"bc{j}",
                                   name=f"bc_{b}{j}")
                bct_by_hic[(h, ic)] = bct
                nc.gpsimd.partition_broadcast(bct[:], rc[:], channels=DV)

        def finish(ic, pv):
            of = out_pool.tile([128, 512], F16, tag="of", name=f"of_{b}{ic}")
            for h in range(HPC):
                nc.vector.tensor_mul(
                    of[h * DV : (h + 1) * DV, :],
                    pv[h * DV : (h + 1) * DV, :],
                    bct_by_hic[(h, ic)][:],
                )
            out_eng.dma_start(
                outcT[:, b * S + ic * 512 : b * S + (ic + 1) * 512], of[:]
            )

        def chain(ic, t8s=range(KT), last=True):
            ex = all_ex[b]
            key = f"pv{ic}"
            if key not in state:
                state[key] = psum_ch.tile(
                    [128, 512], F32, tag="ch", name=f"pv_{b}{ic}")
            pv = state[key]
            for t8 in t8s:
                for h in range(HPC):
                    nc.tensor.matmul(
                        pv[h * DV : (h + 1) * DV, :],
                        vau[t8][:, h * DV : (h + 1) * DV],
                        ex[:, t8, h, ic * 512 : (ic + 1) * 512],
                        start=(t8 == 0),
                        stop=(t8 == KT - 1),
                        skip_group_check=True,
                    )
            if not last:
                return
            while deferred:
                deferred.pop(0)()
            deferred.append(lambda ic=ic, pv=pv: finish(ic, pv))

        def flush_deferred():
            while deferred:
                deferred.pop(0)()

        if not tail:
            chains.append((den_chain, KT * 512))
            for ic in range(NCH):
                chains.append((lambda ic=ic: chain(ic), KT * 512))
            chains.append((flush_deferred, 0))
            if final:
                chains.append((lambda: all_ex.pop(b), 0))
            return chains  # noqa

        # tail mode (last batch): key blocks 0..6 accumulate as soon as their
        # exps land; only the t8=7 matmuls and the drains run after the
        # final ACT, shortening the serial epilogue.
        part = range(KT - 1)
        partials = [
            (lambda: den_chain(part, last=False), (KT - 1) * 512),
            (lambda: chain(0, part, last=False), (KT - 1) * 512),
            (lambda: chain(1, part, last=False), (KT - 1) * 512),
        ]
        finals = [
            (lambda: den_chain(range(KT - 1, KT), last=True), 512),
            (lambda: chain(0, range(KT - 1, KT), last=True), 512),
            (lambda: chain(1, range(KT - 1, KT), last=True), 512),
            (flush_deferred, 0),
        ]
        if final:
            finals.append((lambda: all_ex.pop(b), 0))
        return partials, finals

    def emit_scores_interleaved(b, filler, flush=False):
        """Scores+exp for batch b: per t8 a QUAD of matmuls alternating
        head row groups back-to-back (pairs execute concurrently on the
        PE), then the two ACT exps. `filler` (chain, pe_cycles) entries
        are spread between quads by cycle weight; unconsumed non-strict
        entries are RETURNED so they carry into the next period."""
        fq = list(filler)
        total_w = sum(w for c, w, s in fq) or 1
        done_w = 0.0
        kT = qkvT_by_b[b]["k"]
        qT = qkvT_by_b[b]["q"]
        ex = ex_pool.tile([128, KT, HPC, 1024], F16, tag="ex", name=f"ex_b{b}")
        all_ex[b] = ex
        for t8 in range(KT):
            pss = []
            for h in range(HPC):
                ps = psum_sc.tile([128, 1024], F32, tag="sc2", name=f"sc_{b}{h}{t8}")
                pss.append(ps)
            # quad: (h0,ic0),(h1,ic0),(h0,ic1),(h1,ic1) back-to-back
            for ic in range(NCH):
                for h in range(HPC):
                    hsl = slice(h * DV, (h + 1) * DV)
                    nc.tensor.matmul(
                        pss[h][:, ic * 512 : (ic + 1) * 512],
                        kT[hsl, t8 * 128 : (t8 + 1) * 128],
                        qT[hsl, ic * 512 : (ic + 1) * 512],
                        start=True,
                        stop=True,
                    )
            for h in range(HPC):
                nc.scalar.activation(ex[:, t8, h, :], pss[h][:], EXP, scale=0.125)
            # spread filler chains by PE-cycle weight across the 8 quads
            want = ((t8 + 1) / KT) * total_w
            while fq and done_w < want:
                c, w, strict = fq.pop(0)
                c()
                done_w += w
        # entries marked strict (next batch's q/k projections - consumed by
        # the next period's first quad) may not be carried over
        if flush:
            keep = []
        else:
            keep = [e for e in fq if not e[2]]
        for c, w, strict in fq:
            if flush or strict:
                c()
        return keep

    # ---- prologue -------------------------------------------------------
    # critical path: wq + batch-0 k-pair parts on the sync queue (2KB DRAM
    # lines - the token-sliced variant has 512B lines and less than half
    # the DMA bandwidth); wk in parallel on gpsimd, then wv/biases/ident.
    load_w("q")
    load_w("k")
    sq0 = emit_dma(0, split=True)
    sq1 = emit_dma(1)
    load_w("v")
    id_sb = singles.tile([128, 128], F16, tag="ident", name="id_sb")
    nc.gpsimd.dma_start(id_sb[:], ident[:])

    # Throwaway matmuls to trip the HAM activity monitor while the first
    # seq quarter is still in flight (PE otherwise idles ~2us and then
    # runs the whole projection prologue at the cold 1.2 GHz clock).
    warm = singles.tile([128, 512], F16, tag="warm", name="warm_sb")
    nc.vector.memset(warm[:], 0.0)
    for i in range(18):
        wps = psum_ch.tile([128, 512], F32, tag="ch", name=f"warm{i}")
        nc.tensor.matmul(wps[:], warm[:, 0:128], warm[:], start=True, stop=True)

    # Persistent v tiles ([v_h0 | v_h1] per 128-token block, exactly the
    # paired-transpose output layout), three rotating sets; plus the ones
    # column for the denominator quad.
    ones_sb = singles.tile([128, 1], F16, tag="ones", name="ones_sb")
    nc.gpsimd.memset(ones_sb[:], 1.0)
    va_sets = []
    for s in range(3):
        tiles = []
        for t8 in range(KT):
            va = singles.tile([128, 2 * DV], F16,
                              tag=f"vaug_{s}_{t8}", name=f"vaug_{s}_{t8}")
            tiles.append(va)
        va_sets.append(tiles)

    # q (both chunks) and k's first chunk up front, part-by-part as the seq
    # DMAs land, so scores(0) can start; the first score quad only reads
    # k tokens 0:512, so k's second chunk is deferred into the first
    # period's filler (3 live PSUM tiles: 2 chain ring + 1 score ring).
    qk_ps = {}
    qk_dst = {}
    prologue_sets = [("q", 0), ("q", 1), ("k", 0)]
    for nm in ("q", "k"):
        dst = qkv_pool.tile([128, S], F16, tag=f"{nm}T", name=f"{nm}T_b0")
        qkvT_by_b.setdefault(0, {})[nm] = dst
        qk_dst[nm] = dst
    for nm, ic in prologue_sets:
        qk_ps[(nm, ic)] = psum_ch.tile(
            [128, 512], F32, tag="ch", name=f"qk0_{nm}{ic}")
    # consume parts in their DMA landing order: sync carries parts 0,1 and
    # scalar parts 2,3, so part 1 (serial behind part 0) arrives last
    part_order = (0, 2, 3, 1)
    for j in part_order:
        for nm, ic in prologue_sets:
            for kk in (2 * j, 2 * j + 1):
                nc.tensor.matmul(
                    qk_ps[(nm, ic)][:],
                    w_sb[nm][:, kk, :],
                    sq0[j][:, kk % 2, ic * 512 : (ic + 1) * 512],
                    start=(j == part_order[0] and kk == 2 * j),
                    stop=(j == part_order[-1] and kk == 2 * j + 1),
                )
    for nm, ic in prologue_sets:
        nc.vector.tensor_scalar_add(
            qk_dst[nm][:, ic * 512 : (ic + 1) * 512],
            qk_ps[(nm, ic)][:], b_sb[nm][:])

    def k1_chain():
        """k's second chunk - needed from score quad t8=4 on."""
        def chain():
            ps = psum_ch.tile([128, 512], F32, tag="ch", name="qk0_k1")
            for kk in range(KT):
                nc.tensor.matmul(
                    ps[:],
                    w_sb["k"][:, kk, :],
                    sq0[kk // 2][:, kk % 2, 512:S],
                    start=(kk == 0),
                    stop=(kk == KT - 1),
                )
            nc.vector.tensor_scalar_add(
                qk_dst["k"][:, 512:S], ps[:], b_sb["k"][:])

        return [(chain, KT * 512)]

    def v0_chains():
        return qkv_chains(0, sq0, names=("v",))

    # ---- main pipeline --------------------------------------------------
    def soft(chains):
        return [(c, w, False) for c, w in chains]

    def strict(chains):
        return [(c, w, True) for c, w in chains]

    # Period PE loads:
    #   p0: v(0)+vtr(0) + QKV(1)
    #   p1: vtr(1) + pv(0) + QKV(2)
    #   p2: vtr(2) + pv(1) + QKV(3)
    #   p3: vtr(3) + pv(2)
    #   post: pv(3)
    sq_by_b = {0: sq0, 1: sq1}
    for b in range(B):
        filler = []
        if b == 0:
            filler += strict(k1_chain())
            filler += soft(v0_chains())
            filler += soft(vtr_chains(0))
        if b + 2 < B:
            sq_by_b[b + 2] = emit_dma(b + 2)
        if b + 1 < B:
            filler += strict(qkv_chains(b + 1, sq_by_b[b + 1], names=("q", "k")))
            if b + 1 < B - 1:
                filler += soft(qkv_chains(b + 1, sq_by_b[b + 1], names=("v",)))
        else:
            filler += soft(qkv_chains(b, sq_by_b[b], names=("v",)))
        if b >= 1:
            pvc = pv_chains(b - 1, final=True)
            filler.insert(0, (pvc[0][0], pvc[0][1], False))  # den chain first
            pv_rest = soft(pvc[1:])
        else:
            pv_rest = []
        if b + 1 < B - 1:
            filler += soft(vtr_chains(b + 1))
        if b == B - 1:
            filler += soft(vtr_chains(b))
        filler += pv_rest
        emit_scores_interleaved(b, filler, flush=True)
    for c, w in pv_chains(B - 1, final=True):
        c()


def _build():
    if "nc" in _CACHE:
        return _CACHE["nc"]
    nc = bacc.Bacc(
        "TRN2",
        target_bir_lowering=False,
        debug=False,
        enable_asserts=False,
        num_devices=N_CORES,
    )
    seqT = nc.dram_tensor("seqT", [128, B, KT, S], F16, kind="ExternalInput").ap()
    wT = {
        name: nc.dram_tensor(f"w{name}T", [128, KT, DPC], F16, kind="ExternalInput").ap()
        for name in ("q", "k", "v")
    }
    bias = {
        name: nc.dram_tensor(f"b{name}", [DPC, 1], F32, kind="ExternalInput").ap()
        for name in ("q", "k", "v")
    }
    ident = nc.dram_tensor("ident", [128, 128], F16, kind="ExternalInput").ap()
    outcT = nc.dram_tensor("outcT", [HPC * DV, B * S], F16, kind="ExternalOutput").ap()

    with tile.TileContext(nc) as tc:
        with ExitStack() as ctx:
            _emit(ctx, tc, seqT, wT, bias, ident, outcT)
    nc.compile()
    _CACHE["nc"] = nc
    return nc


def make_in_maps(seq, Wq, bq, Wk, bk, Wv, bv):
    f16 = np.float16
    # [p, b, k, tok]: a k-pair part of one batch is 4KB contiguous per
    # partition, which roughly doubles realized DMA bandwidth vs 2KB lines
    seqT_full = np.ascontiguousarray(
        np.asarray(seq).transpose(2, 0, 1).reshape(KT, 128, B, S)
        .transpose(1, 2, 0, 3).astype(f16)
    )
    ident = np.eye(128, dtype=f16)

    def wtile(W, sl):
        # W[sl].T is [d_in, 128] -> [p, k, 128]
        return np.ascontiguousarray(
            np.asarray(W)[sl].T.reshape(KT, 128, DPC).transpose(1, 0, 2).astype(f16)
        )

    in_maps = []
    for c in range(N_CORES):
        sl = slice(c * DPC, (c + 1) * DPC)
        in_maps.append(
            {
                "seqT": seqT_full,
                "wqT": wtile(Wq, sl),
                "wkT": wtile(Wk, sl),
                "wvT": wtile(Wv, sl),
                "bq": np.ascontiguousarray(
                    np.asarray(bq, np.float32)[sl].reshape(DPC, 1)
                ),
                "bk": np.ascontiguousarray(
                    np.asarray(bk, np.float32)[sl].reshape(DPC, 1)
                ),
                "bv": np.ascontiguousarray(
                    np.asarray(bv, np.float32)[sl].reshape(DPC, 1)
                ),
                "ident": ident,
            }
        )
    return in_maps


def assemble(results):
    """[cores][h*64+d, b*1024+i] -> [B, S, D]"""
    out = np.empty((B, S, D), np.float32)
    for c in range(N_CORES):
        r = results[c]["outcT"].astype(np.float32).reshape(DPC, B, S)  # [hd, b, i]
        out[:, :, c * DPC : (c + 1) * DPC] = r.transpose(1, 2, 0)
    return out


def kernel(seq, Wq, bq, Wk, bk, Wv, bv):
    global LAST_RESULTS
    nc = _build()
    in_maps = make_in_maps(seq, Wq, bq, Wk, bk, Wv, bv)
    res = run_bass_kernel_spmd(
        nc, in_maps, core_ids=list(range(N_CORES)), trace=TRACE, **TRACE_KWARGS
    )
    LAST_RESULTS = res
    return assemble(res.results)


# revision 45
# speedup vs baseline: 1.0129x; 1.0129x over previous
"""BERT self-attention (B=4, S=1024, D=1024, H=16) on 8 TRN2 NeuronCores.

Sharding: tensor-parallel over heads. Core c owns output dims
[c*128, (c+1)*128) of Wq/Wk/Wv (= heads 2c and 2c+1) and computes those
heads' attention for all 4 batches. seq is replicated; the host pre-tiles
seqT -> [128, B, KT, S] fp16 (a k-pair part of one batch is 4KB
contiguous per partition - the widest DMA lines this layout allows) and
weight shards -> [128, KT, 128].

Per-core pipeline (per batch):
  qT/kT/vT [128, S] = W_shard @ seqT_b        (K=1024, N=512 chunks)
  v token-major via ONE 128-wide PE transpose per key block (both heads
      at once) into [v_h0 | v_h1] tiles - exactly the transpose output
      layout, so the drain is a single DVE copy
  scores: per t8 (128-key block) a QUAD of matmuls alternating head row
      groups (h0 rows 0:64 / h1 rows 64:128) back-to-back, so the PE
      runs both heads' K=64 matmuls concurrently in disjoint row tiles
      (2x throughput vs sequential).
      scoresT[j,i] = k_j . q_i  ->  expT = exp(0.125*scoresT)  (ACT)
      written into the per-batch ex tile [128, KT, HPC, 1024] fp16.
  p@v: per chunk BOTH heads run concurrently as M=64 matmuls in
      disjoint PE column groups (cols 0:64 / 64:128) accumulating into
      one [128, 512] PSUM tile (col-tiling pairs small-M matmuls the
      same way row-tiling pairs the score quads).
  softmax denominators: a 4x column-tiled quad of M=1 ones-matmuls
      (one per (head, chunk), output partitions 0/32/64/96 of one PSUM
      bank) accumulated over key blocks; drained by per-(head,chunk)
      DVE copy + reciprocal and a GpSimd partition-broadcast, all
      emitted at PERIOD START so the broadcasts are long done when the
      p@v multiplies need them. The final multiply reads the p@v tile
      straight from PSUM, deferred one chain so the broadcast wait
      never blocks the next chain's PSUM release; fp16 DMA out.

Schedule (the ACT engine carries ~68us of exp; the PE ~107us of
matmul+ldweights, so the PE is the pacer and must never stall):
  - per-BATCH seq DMA, two batches prefetched ahead (batch 0 split
    across the sync and scalar DMA rings), so projection chains never
    wait on DMA mid-period;
  - 18 throwaway matmuls at t=0 trip the HAM activity monitor toward
    the 2.4 GHz clock while batch 0's seq is still in flight;
  - batch-0 q + k(first chunk) accumulate part-by-part as the DMAs
    land; k's second chunk (not needed until score quad t8=4) is
    deferred into the first period's filler, shortening the path to
    the first exp;
  - all non-score PE work is emitted as consecutive accumulation
    chains spread between score quads by cycle weight; periods carry
    {next batch's QKV, this batch's v-transposes, previous batch's
    p@v} so each stays just above the ACT pace.

HW-validated pitfalls baked in: DVE reciprocal must read SBUF at
partition 0 (PSUM or offset-partition inputs give garbage); GpSimd
partition_broadcast only handles base-partition-0 source AND
destination; DVE access patterns cannot stride the partition dim; each
partition-disjoint accumulation group needs its own start=True (the
has_written clear is per-partition).

The host transposes the gathered [head, d, token] result back to
[token, d] (layout only - all FLOPs happen on-device).

The softmax skips the max-subtraction: scores ~ N(0,1) so exp() is
comfortably in fp32 range, and exp(x)/sum(exp(x)) is algebraically
identical to the max-shifted form.
"""

import numpy as np
from contextlib import ExitStack

import concourse.bass as bass
import concourse.tile as tile
from concourse import bacc, mybir
from concourse.bass_utils import run_bass_kernel_spmd

N_CORES = 8
B, S, D = 4, 1024, 1024
DPC = 128  # output dims per core (2 heads x 64)
HPC = 2  # heads per core
DV = 64  # head dim
KT = D // 128  # contraction tiles
NCH = S // 512  # 512-wide free-dim chunks per batch
F32 = mybir.dt.float32
F16 = mybir.dt.float16
EXP = mybir.ActivationFunctionType.Exp

# test.py may flip these to profile; the grading path leaves them alone.
TRACE = False
TRACE_KWARGS = {}
LAST_RESULTS = None

_CACHE = {}


def _emit(ctx, tc, seqT, wT, bias, ident, outcT):
    nc = tc.nc

    singles = ctx.enter_context(tc.tile_pool(name="singles", bufs=1))
    seq_pool = ctx.enter_context(tc.tile_pool(name="seq", bufs=3))
    qkv_pool = ctx.enter_context(tc.tile_pool(name="qkv", bufs=2))
    ex_pool = ctx.enter_context(tc.tile_pool(name="expT", bufs=2))
    small_pool = ctx.enter_context(tc.tile_pool(name="small", bufs=2))
    bc_pool = ctx.enter_context(tc.tile_pool(name="bc", bufs=2))
    out_pool = ctx.enter_context(tc.tile_pool(name="out", bufs=2))
    psum_ch = ctx.enter_context(tc.tile_pool(name="psum_ch", bufs=3, space="PSUM"))
    psum_sc = ctx.enter_context(tc.tile_pool(name="psum_sc", bufs=2, space="PSUM"))
    psum_den = ctx.enter_context(tc.tile_pool(name="psum_den", bufs=1, space="PSUM"))

    w_sb = {}
    b_sb = {}

    def load_w(name, eng=None):
        # one DMA per weight: DRAM [128, KT, 128] -> SBUF [128, KT, 128]
        eng = eng if eng is not None else nc.gpsimd
        wt = singles.tile([128, KT, 128], F16, tag=f"w{name}", name=f"w{name}_sb")
        eng.dma_start(wt[:], wT[name][:])
        w_sb[name] = wt
        bt = singles.tile([128, 1], F32, tag=f"b{name}", name=f"b{name}_sb")
        nc.gpsimd.dma_start(bt[:], bias[name][:])
        b_sb[name] = bt

    all_ex = {}
    qkvT_by_b = {}

    def alloc_seq(b):
        # 4 sub-tiles of 2 k-tiles each so the first QKV matmuls only wait
        # on a quarter of the batch's k-planes
        return [
            seq_pool.tile([128, 2, S], F16, tag=f"seqT{j}", name=f"seqT_b{b}p{j}")
            for j in range(4)
        ]

    def emit_dma(b, split=False):
        sq = alloc_seq(b)
        for j in range(4):
            # scalar's DMA ring is idle in the prologue; splitting batches
            # 0/1 across two rings halves their latency. Later batches stay
            # off scalar so DMA issue never delays an ACT.
            eng = nc.scalar if split and j >= 2 else nc.sync
            eng.dma_start(sq[j][:], seqT[:, b, 2 * j : 2 * j + 2, :])
        return sq

    def qkv_chains(b, sq, names=("q", "k", "v")):
        """One chain per (name, chunk): 8 consecutive matmuls accumulating
        K=1024 into one PSUM tile, then a DVE bias-add drain."""
        chains = []
        dsts = qkvT_by_b.setdefault(b, {})
        for name in names:
            dst = qkv_pool.tile([128, S], F16, tag=f"{name}T", name=f"{name}T_b{b}")
            dsts[name] = dst
            for ic in range(NCH):

                def chain(name=name, ic=ic, dst=dst):
                    ps = psum_ch.tile([128, 512], F32, tag="ch", name=f"mm_{name}{b}{ic}")
                    for kk in range(KT):
                        nc.tensor.matmul(
                            ps[:],
                            w_sb[name][:, kk, :],
                            sq[kk // 2][:, kk % 2, ic * 512 : (ic + 1) * 512],
                            start=(kk == 0),
                            stop=(kk == KT - 1),
                        )
                    nc.vector.tensor_scalar_add(
                        dst[:, ic * 512 : (ic + 1) * 512], ps[:], b_sb[name][:]
                    )

                chains.append((chain, KT * 512))
        return chains

    def vtr_chains(b):
        """v (token-major) via ONE 128-wide PE transpose per key block
        (both heads at once); DVE drains into the ones-augmented tiles."""
        chains = []
        vau = va_sets[b % 3]
        for t8 in range(KT):

            def tr(t8=t8, vau=vau):
                vT = qkvT_by_b[b]["v"]
                va = vau[t8]
                pt = psum_ch.tile([128, 128], F16, tag="ch", name=f"vtr_{b}{t8}")
                nc.tensor.transpose(
                    pt[:], vT[:, t8 * 128 : (t8 + 1) * 128], id_sb[:]
                )
                nc.vector.tensor_copy(va[:], pt[:])

            chains.append((tr, 192))
        return chains

    def pv_chains(b, final=False, tail=False):
        """Per chunk, BOTH heads' p@v run concurrently as M=64 matmuls in
        disjoint PE column groups (cols 0:64 / 64:128) accumulating into one
        [128, 512] PSUM tile. The softmax denominators come from a separate
        4x column-tiled quad of M=1 ones-matmuls (one per (head, chunk), at
        output partitions 0/32/64/96 of one PSUM bank), drained by 4 ACT
        copies (ScalarE has slack), one DVE reciprocal over [4, 512], and a
        GpSimd partition-broadcast per (head, chunk). The final multiply is
        one [128, 512] DVE op reading the PV tile straight from PSUM,
        deferred one chain so the broadcast wait never blocks the next
        chain's PSUM release; fp16 DMA out."""
        chains = []
        vau = va_sets[b % 3]
        deferred = []
        out_eng = nc.sync if b == B - 1 else nc.gpsimd
        HIC = [(0, 0), (1, 0), (0, 1), (1, 1)]
        bct_by_hic = {}

        state = {}

        def den_chain(t8s=range(KT), last=True):
            ex = all_ex[b]
            if "dps" not in state:
                state["dps"] = psum_den.tile(
                    [128, 512], F32, tag="den", name=f"denps_{b}")
            dps = state["dps"]
            for t8 in t8s:
                for j, (h, ic) in enumerate(HIC):
                    nc.tensor.matmul(
                        dps[32 * j : 32 * j + 1, :],
                        ones_sb[:, 0:1],
                        ex[:, t8, h, ic * 512 : (ic + 1) * 512],
                        start=(t8 == 0),
                        stop=(t8 == KT - 1),
                        skip_group_check=True,
                        tile_position=(0, 32 * j),
                    )
            if not last:
                return
            for j, (h, ic) in enumerate(HIC):
                den_sb = small_pool.tile([1, 512], F32, tag=f"den{j}",
                                         name=f"den_{b}{j}")
                if j == 0:
                    # partition-0 PSUM read is legal on ACT (unlike 32/64/96);
                    # offloading the first copy shortens the serial DVE drain
                    nc.scalar.copy(den_sb[:], dps[0:1, :])
                else:
                    nc.vector.tensor_copy(den_sb[:], dps[32 * j : 32 * j + 1, :])
                rc = small_pool.tile([1, 512], F32, tag=f"recip{j}",
                                     name=f"rc_{b}{j}")
                nc.vector.reciprocal_approx_fast(rc[:], den_sb[:])
                bct = bc_pool.tile([DV, 512], F32, tag=f"bc{j}",
                                   name=f"bc_{b}{j}")
                bct_by_hic[(h, ic)] = bct
                nc.gpsimd.partition_broadcast(bct[:], rc[:], channels=DV)

        def finish(ic, pv):
            of = out_pool.tile([128, 512], F16, tag="of", name=f"of_{b}{ic}")
            for h in range(HPC):
                nc.vector.tensor_mul(
                    of[h * DV : (h + 1) * DV, :],
                    pv[h * DV : (h + 1) * DV, :],
                    bct_by_hic[(h, ic)][:],
                )
            out_eng.dma_start(
                outcT[:, b * S + ic * 512 : b * S + (ic + 1) * 512], of[:]
            )

        def chain(ic, t8s=range(KT), last=True):
            ex = all_ex[b]
            key = f"pv{ic}"
            if key not in state:
                state[key] = psum_ch.tile(
                    [128, 512], F32, tag="ch", name=f"pv_{b}{ic}")
            pv = state[key]
            for t8 in t8s:
                for h in range(HPC):
                    nc.tensor.matmul(
                        pv[h * DV : (h + 1) * DV, :],
                        vau[t8][:, h * DV : (h + 1) * DV],
                        ex[:, t8, h, ic * 512 : (ic + 1) * 512],
                        start=(t8 == 0),
                        stop=(t8 == KT - 1),
                        skip_group_check=True,
                    )
            if not last:
                return
            while deferred:
                deferred.pop(0)()
            deferred.append(lambda ic=ic, pv=pv: finish(ic, pv))

        def flush_deferred():
            while deferred:
                deferred.pop(0)()

        if not tail:
            chains.append((den_chain, KT * 512))
            for ic in range(NCH):
                chains.append((lambda ic=ic: chain(ic), KT * 512))
            chains.append((flush_deferred, 0))
            if final:
                chains.append((lambda: all_ex.pop(b), 0))
            return chains  # noqa

        # tail mode (last batch): key blocks 0..6 accumulate as soon as their
        # exps land; only the t8=7 matmuls and the drains run after the
        # final ACT, shortening the serial epilogue.
        part = range(KT - 1)
        partials = [
            (lambda: den_chain(part, last=False), (KT - 1) * 512),
            (lambda: chain(0, part, last=False), (KT - 1) * 512),
            (lambda: chain(1, part, last=False), (KT - 1) * 512),
        ]
        finals = [
            (lambda: den_chain(range(KT - 1, KT), last=True), 512),
            (lambda: chain(0, range(KT - 1, KT), last=True), 512),
            (lambda: chain(1, range(KT - 1, KT), last=True), 512),
            (flush_deferred, 0),
        ]
        if final:
            finals.append((lambda: all_ex.pop(b), 0))
        return partials, finals

    def emit_scores_interleaved(b, filler, flush=False):
        """Scores+exp for batch b: per t8 a QUAD of matmuls alternating
        head row groups back-to-back (pairs execute concurrently on the
        PE), then the two ACT exps. `filler` (chain, pe_cycles) entries
        are spread between quads by cycle weight; unconsumed non-strict
        entries are RETURNED so they carry into the next period."""
        fq = list(filler)
        total_w = sum(w for c, w, s in fq) or 1
        done_w = 0.0
        kT = qkvT_by_b[b]["k"]
        qT = qkvT_by_b[b]["q"]
        ex = ex_pool.tile([128, KT, HPC, 1024], F16, tag="ex", name=f"ex_b{b}")
        all_ex[b] = ex
        for t8 in range(KT):
            pss = []
            for h in range(HPC):
                ps = psum_sc.tile([128, 1024], F32, tag="sc2", name=f"sc_{b}{h}{t8}")
                pss.append(ps)
            # quad: (h0,ic0),(h1,ic0),(h0,ic1),(h1,ic1) back-to-back
            for ic in range(NCH):
                for h in range(HPC):
                    hsl = slice(h * DV, (h + 1) * DV)
                    nc.tensor.matmul(
                        pss[h][:, ic * 512 : (ic + 1) * 512],
                        kT[hsl, t8 * 128 : (t8 + 1) * 128],
                        qT[hsl, ic * 512 : (ic + 1) * 512],
                        start=True,
                        stop=True,
                    )
            for h in range(HPC):
                nc.scalar.activation(ex[:, t8, h, :], pss[h][:], EXP, scale=0.125)
            # spread filler chains by PE-cycle weight across the 8 quads
            want = ((t8 + 1) / KT) * total_w
            while fq and done_w < want:
                c, w, strict = fq.pop(0)
                c()
                done_w += w
        # entries marked strict (next batch's q/k projections - consumed by
        # the next period's first quad) may not be carried over
        if flush:
            keep = []
        else:
            keep = [e for e in fq if not e[2]]
        for c, w, strict in fq:
            if flush or strict:
                c()
        return keep

    # ---- prologue -------------------------------------------------------
    # critical path: wq + batch-0 k-pair parts on the sync queue (2KB DRAM
    # lines - the token-sliced variant has 512B lines and less than half
    # the DMA bandwidth); wk in parallel on gpsimd, then wv/biases/ident.
    load_w("q")
    load_w("k")
    sq0 = emit_dma(0, split=True)
    sq1 = emit_dma(1)
    load_w("v")
    id_sb = singles.tile([128, 128], F16, tag="ident", name="id_sb")
    nc.gpsimd.dma_start(id_sb[:], ident[:])

    # Throwaway matmuls to trip the HAM activity monitor while the first
    # seq quarter is still in flight (PE otherwise idles ~2us and then
    # runs the whole projection prologue at the cold 1.2 GHz clock).
    warm = singles.tile([128, 512], F16, tag="warm", name="warm_sb")
    nc.vector.memset(warm[:], 0.0)
    for i in range(22):
        wps = psum_ch.tile([128, 512], F32, tag="ch", name=f"warm{i}")
        nc.tensor.matmul(wps[:], warm[:, 0:128], warm[:], start=True, stop=True)

    # Persistent v tiles ([v_h0 | v_h1] per 128-token block, exactly the
    # paired-transpose output layout), three rotating sets; plus the ones
    # column for the denominator quad.
    ones_sb = singles.tile([128, 1], F16, tag="ones", name="ones_sb")
    nc.gpsimd.memset(ones_sb[:], 1.0)
    va_sets = []
    for s in range(3):
        tiles = []
        for t8 in range(KT):
            va = singles.tile([128, 2 * DV], F16,
                              tag=f"vaug_{s}_{t8}", name=f"vaug_{s}_{t8}")
            tiles.append(va)
        va_sets.append(tiles)

    # q (both chunks) and k's first chunk up front, part-by-part as the seq
    # DMAs land, so scores(0) can start; the first score quad only reads
    # k tokens 0:512, so k's second chunk is deferred into the first
    # period's filler (3 live PSUM tiles: 2 chain ring + 1 score ring).
    qk_ps = {}
    qk_dst = {}
    prologue_sets = [("q", 0), ("q", 1), ("k", 0)]
    for nm in ("q", "k"):
        dst = qkv_pool.tile([128, S], F16, tag=f"{nm}T", name=f"{nm}T_b0")
        qkvT_by_b.setdefault(0, {})[nm] = dst
        qk_dst[nm] = dst
    for nm, ic in prologue_sets:
        qk_ps[(nm, ic)] = psum_ch.tile(
            [128, 512], F32, tag="ch", name=f"qk0_{nm}{ic}")
    # consume parts in their DMA landing order: sync carries parts 0,1 and
    # scalar parts 2,3, so part 1 (serial behind part 0) arrives last
    part_order = (0, 2, 3, 1)
    for j in part_order:
        for nm, ic in prologue_sets:
            for kk in (2 * j, 2 * j + 1):
                nc.tensor.matmul(
                    qk_ps[(nm, ic)][:],
                    w_sb[nm][:, kk, :],
                    sq0[j][:, kk % 2, ic * 512 : (ic + 1) * 512],
                    start=(j == part_order[0] and kk == 2 * j),
                    stop=(j == part_order[-1] and kk == 2 * j + 1),
                )
    for nm, ic in prologue_sets:
        nc.vector.tensor_scalar_add(
            qk_dst[nm][:, ic * 512 : (ic + 1) * 512],
            qk_ps[(nm, ic)][:], b_sb[nm][:])

    def k1_chain():
        """k's second chunk - needed from score quad t8=4 on."""
        def chain():
            ps = psum_ch.tile([128, 512], F32, tag="ch", name="qk0_k1")
            for kk in range(KT):
                nc.tensor.matmul(
                    ps[:],
                    w_sb["k"][:, kk, :],
                    sq0[kk // 2][:, kk % 2, 512:S],
                    start=(kk == 0),
                    stop=(kk == KT - 1),
                )
            nc.vector.tensor_scalar_add(
                qk_dst["k"][:, 512:S], ps[:], b_sb["k"][:])

        return [(chain, KT * 512)]

    def v0_chains():
        return qkv_chains(0, sq0, names=("v",))

    # ---- main pipeline --------------------------------------------------
    def soft(chains):
        return [(c, w, False) for c, w in chains]

    def strict(chains):
        return [(c, w, True) for c, w in chains]

    # Period PE loads:
    #   p0: v(0)+vtr(0) + QKV(1)
    #   p1: vtr(1) + pv(0) + QKV(2)
    #   p2: vtr(2) + pv(1) + QKV(3)
    #   p3: vtr(3) + pv(2)
    #   post: pv(3)
    sq_by_b = {0: sq0, 1: sq1}
    for b in range(B):
        filler = []
        if b == 0:
            filler += strict(k1_chain())
            filler += soft(v0_chains())
            filler += soft(vtr_chains(0))
        if b + 2 < B:
            sq_by_b[b + 2] = emit_dma(b + 2)
        if b + 1 < B:
            filler += strict(qkv_chains(b + 1, sq_by_b[b + 1], names=("q", "k")))
            if b + 1 < B - 1:
                filler += soft(qkv_chains(b + 1, sq_by_b[b + 1], names=("v",)))
        else:
            filler += soft(qkv_chains(b, sq_by_b[b], names=("v",)))
        if b >= 1:
            pvc = pv_chains(b - 1, final=True)
            filler.insert(0, (pvc[0][0], pvc[0][1], False))  # den chain first
            pv_rest = soft(pvc[1:])
        else:
            pv_rest = []
        if b + 1 < B - 1:
            filler += soft(vtr_chains(b + 1))
        if b == B - 1:
            filler += soft(vtr_chains(b))
        filler += pv_rest
        emit_scores_interleaved(b, filler, flush=True)
    for c, w in pv_chains(B - 1, final=True):
        c()


def _build():
    if "nc" in _CACHE:
        return _CACHE["nc"]
    nc = bacc.Bacc(
        "TRN2",
        target_bir_lowering=False,
        debug=False,
        enable_asserts=False,
        num_devices=N_CORES,
    )
    seqT = nc.dram_tensor("seqT", [128, B, KT, S], F16, kind="ExternalInput").ap()
    wT = {
        name: nc.dram_tensor(f"w{name}T", [128, KT, DPC], F16, kind="ExternalInput").ap()
        for name in ("q", "k", "v")
    }
    bias = {
        name: nc.dram_tensor(f"b{name}", [DPC, 1], F32, kind="ExternalInput").ap()
        for name in ("q", "k", "v")
    }
    ident = nc.dram_tensor("ident", [128, 128], F16, kind="ExternalInput").ap()
    outcT = nc.dram_tensor("outcT", [HPC * DV, B * S], F16, kind="ExternalOutput").ap()

    with tile.TileContext(nc) as tc:
        with ExitStack() as ctx:
            _emit(ctx, tc, seqT, wT, bias, ident, outcT)
    nc.compile()
    _CACHE["nc"] = nc
    return nc


def make_in_maps(seq, Wq, bq, Wk, bk, Wv, bv):
    f16 = np.float16
    # [p, b, k, tok]: a k-pair part of one batch is 4KB contiguous per
    # partition, which roughly doubles realized DMA bandwidth vs 2KB lines
    seqT_full = np.ascontiguousarray(
        np.asarray(seq).transpose(2, 0, 1).reshape(KT, 128, B, S)
        .transpose(1, 2, 0, 3).astype(f16)
    )
    ident = np.eye(128, dtype=f16)

    def wtile(W, sl):
        # W[sl].T is [d_in, 128] -> [p, k, 128]
        return np.ascontiguousarray(
            np.asarray(W)[sl].T.reshape(KT, 128, DPC).transpose(1, 0, 2).astype(f16)
        )

    in_maps = []
    for c in range(N_CORES):
        sl = slice(c * DPC, (c + 1) * DPC)
        in_maps.append(
            {
                "seqT": seqT_full,
                "wqT": wtile(Wq, sl),
                "wkT": wtile(Wk, sl),
                "wvT": wtile(Wv, sl),
                "bq": np.ascontiguousarray(
                    np.asarray(bq, np.float32)[sl].reshape(DPC, 1)
                ),
                "bk": np.ascontiguousarray(
                    np.asarray(bk, np.float32)[sl].reshape(DPC, 1)
                ),
                "bv": np.ascontiguousarray(
                    np.asarray(bv, np.float32)[sl].reshape(DPC, 1)
                ),
                "ident": ident,
            }
        )
    return in_maps


def assemble(results):
    """[cores][h*64+d, b*1024+i] -> [B, S, D]"""
    out = np.empty((B, S, D), np.float32)
    for c in range(N_CORES):
        r = results[c]["outcT"].astype(np.float32).reshape(DPC, B, S)  # [hd, b, i]
        out[:, :, c * DPC : (c + 1) * DPC] = r.transpose(1, 2, 0)
    return out


def kernel(seq, Wq, bq, Wk, bk, Wv, bv):
    global LAST_RESULTS
    nc = _build()
    in_maps = make_in_maps(seq, Wq, bq, Wk, bk, Wv, bv)
    res = run_bass_kernel_spmd(
        nc, in_maps, core_ids=list(range(N_CORES)), trace=TRACE, **TRACE_KWARGS
    )
    LAST_RESULTS = res
    return assemble(res.results)
